# revision 19
# baseline (speedup 1.0000x reference)
"""Bass/Tile kernel for nn_Att_28879360099124 on 8 TRN2 NeuronCores.

Computes, for full inputs
    hiddenState [TQ=1024, B=16, H=1024] f32
    encoderOut  [S=4096,  B=16, H=1024] f32
the reference
    scores = einsum('sbh,tbh->bst')          # [B, S, TQ]
    attW   = softmax(tanh(scores), axis=S)   # [B, S, TQ]

Strategy: data-parallel over B (2 batches per core, no communication).
All device traffic is bf16 (inputs cast on host, output stored bf16 and
upcast on host): 36MB/core vs 72MB in fp32, taking DMA off the critical
path (per-core HBM is ~358 GB/s; in fp32 the DMA engines were ~65% busy
and co-limiting).  Matmul rate is identical for bf16 and fp32r (1
column/cycle at 2.4GHz) but bf16 enables FWL weight loads: measured MM
spacing drops 227ns -> 216ns = the N/2.4GHz + 2.5ns NX floor.  A
1024-column moving operand is ISA-illegal (s3d3_mm_num_elements), so
matmuls stay at N=512.  Whole-pipeline bf16 rel-err ~5.8e-3 against the
2e-2 budget.

Per core, per batch b:
  - score tiles are [t_p=128, s_f] so the softmax axis (s) is the free dim.
  - matmul: psum[t128, s512] += hidT[h128, t128].T @ encT[h128, s512],
    accumulated over 8 h-tiles.
  - ACT: tanh in-place on psum, then exp psum->SBUF (bf16) with accum_out
    giving the per-t partial row sum of each s-block for free.
  - DVE: reduce partials, reciprocal, per-partition scale to bf16 (DVE
    only -- Pool tensor ops are Q7-software slow, ~7.5us per block); out
    via gpsimd (SWDGE) so stores never block input loads on the Sync
    queue.
Both batches' encoder tiles are SBUF-resident in bf16 (16 x 8KB/partition
= 128KB), as are all 16 hid tiles (32KB); everything is prefetched on the
sync queue up front (no batch-flip stall).  8 dummy matmuls on a memset
scratch tile pre-warm the HAM clock gate to 8/8 during the ~11us of
preamble + first-DMA latency.  The first NCHASE=2 t-tiles run a fused
quarter-major "chase" interleaved with the enc arrival stream.  The final
t-tile runs its last s-block in a private psum tile (tile-granular WAR
tracking would otherwise serialize s-block 6's ACT behind s-block 7's
matmuls) and its stores avoid gpsimd (slow Q7 completion drain),
alternating the sync + scalar HWDGE queues instead.

Measured 244.9-245.3us/core (baseline fp32r kernel: 288.6us): ~11.6us
preamble+first-load latency, 221.2us matmul floor + ~3us periodic
timer-tick stalls, ~10us tail (final tanh/exp chain, scale+store fan,
end-of-NEFF barrier).
"""

import numpy as np

TQ, B, H, S = 1024, 16, 1024, 4096
NCORES = 8
B_LOC = B // NCORES  # batches per core
P = 128
HT = H // P          # 8 h-tiles
TT = TQ // P         # 8 t-tiles per batch
SBLK = 512           # one PSUM bank of fp32
NSB = S // SBLK      # 8 s-blocks
MBLK = 1024          # matmul moving free dim (bf16 max), 2 PSUM banks
NMB = S // MBLK      # 4 moving blocks
NCHASE = 2           # t-tiles fused into the enc-arrival chase
Q = S // 4           # enc b0 load chunk: quarter of the s axis

_CACHE = {}


def _build():
    import concourse.bacc as bacc
    import concourse.mybir as mybir
    import concourse.tile as tile

    f32 = mybir.dt.float32
    bf16 = mybir.dt.bfloat16
    Act = mybir.ActivationFunctionType

    nc = bacc.Bacc("TRN2", target_bir_lowering=False, debug=False,
                   num_devices=NCORES)

    # hid is host-pretiled to [b, ti, hp, hi, t] so each partition's load
    # is one contiguous 2KB run.
    hid_d = nc.dram_tensor("hidT", [B_LOC, TT, P, HT, P], bf16,
                           kind="ExternalInput").ap()
    enc_d = nc.dram_tensor("encT", [B_LOC, HT, P, S], bf16,
                           kind="ExternalInput").ap()
    out_d = nc.dram_tensor("attW", [B_LOC, TT, P, S], bf16,
                           kind="ExternalOutput").ap()

    with tile.TileContext(nc) as tc:
        with (
            tc.tile_pool(name="encp", bufs=B_LOC * HT) as encp,
            tc.tile_pool(name="hidp", bufs=B_LOC * TT) as hidp,
            tc.tile_pool(name="expp", bufs=3) as expp,
            tc.tile_pool(name="smallp", bufs=4) as smallp,
            tc.tile_pool(name="warmp", bufs=1) as warmp,
            tc.tile_pool(name="psum", bufs=4, space="PSUM") as psump,
        ):
            # ---- PE pre-warm: dummy matmuls on a memset scratch tile so
            # the HAM clock gate reaches 8/8 before the first real matmul
            # (which is gated by the first DMA arrivals at ~9-10us).
            warm_t = warmp.tile([P, 640], bf16, name="warm", tag="warm")
            nc.vector.memset(warm_t, 0)
            warm_ps = psump.tile([P, 2, SBLK], f32, name="warm_ps", tag="ps")
            for _ in range(8):
                nc.tensor.matmul(warm_ps[:, 0], lhsT=warm_t[:, 0:P],
                                 rhs=warm_t[:, P:P + SBLK],
                                 start=True, stop=True)

            hid_tiles = {}
            enc_tiles = {}
            for b in range(B_LOC):
                for hi in range(HT):
                    enc_tiles[b, hi] = encp.tile([P, S], bf16,
                                                 name=f"enc_{b}_{hi}",
                                                 tag="enc")

            def load_hid(b, ti):
                hid_t = hidp.tile([P, HT, P], bf16, name=f"hid_{b}_{ti}",
                                  tag="hid")
                nc.sync.dma_start(out=hid_t, in_=hid_d[b, ti])
                hid_tiles[b, ti] = hid_t

            # ---- DMA program order (sync queue is FIFO). The first MM
            # needs hid(0,0) + enc(0,h0,q0), so those are triggers #1/#2.
            # (Splitting these into smaller head pieces was tried and is
            # a net loss: each extra trigger costs ~650ns issue + ~2.5us
            # to its completion semaphore, which starves the chase's
            # first matmuls.)
            load_hid(0, 0)
            nc.sync.dma_start(out=enc_tiles[0, 0][:, 0:Q],
                              in_=enc_d[0, 0, :, 0:Q])
            load_hid(0, 1)
            for hi in range(1, HT):
                nc.sync.dma_start(out=enc_tiles[0, hi][:, 0:Q],
                                  in_=enc_d[0, hi, :, 0:Q])
            for q in range(1, 4):
                for hi in range(HT):
                    nc.sync.dma_start(
                        out=enc_tiles[0, hi][:, q * Q:(q + 1) * Q],
                        in_=enc_d[0, hi, :, q * Q:(q + 1) * Q])
            for ti in range(NCHASE, TT):
                load_hid(0, ti)
            for ti in range(TT):
                load_hid(1, ti)
            for hi in range(HT):
                # batch 1 enc: no chase needed, full-tile loads (1MB)
                nc.sync.dma_start(out=enc_tiles[1, hi], in_=enc_d[1, hi])

            def finalize(b, ti, exp_row, partials, n_acc, mode):
                sums = smallp.tile([P, 1], f32, name=f"sum_{b}_{ti}",
                                   tag="sums")
                nc.vector.reduce_sum(out=sums, in_=partials[:, :n_acc],
                                     axis=mybir.AxisListType.X)
                recip = smallp.tile([P, 1], f32, name=f"rcp_{b}_{ti}",
                                    tag="recip")
                nc.vector.reciprocal(out=recip, in_=sums)
                # All scales on DVE (fast: ~0.5us per 2-block pair; the
                # Pool engine's tensor ops are Q7-software slow, ~7.5us).
                # Steady tiles store via gpsimd (SWDGE) so stores never
                # contend with input loads on the sync queue; the last two
                # tiles avoid gpsimd (its Q7 completion drain is slow and
                # would push the final barrier out) and alternate the
                # sync + scalar HWDGE queues instead.
                for sc in range(0, NSB, 2):
                    nc.vector.tensor_scalar_mul(
                        exp_row[:, sc:sc + 2],
                        exp_row[:, sc:sc + 2], recip)
                    eng = (nc.gpsimd if mode == "steady"
                           else (nc.sync if sc % 4 == 0 else nc.scalar))
                    eng.dma_start(
                        out=out_d[b, ti, :, sc * SBLK:(sc + 2) * SBLK],
                        in_=exp_row[:, sc:sc + 2])

            # ---- fused quarter-major chase over the first t-tiles ----
            chase_exp = [expp.tile([P, NSB, SBLK], bf16,
                                   name=f"exp_0_{j}", tag="exp")
                         for j in range(NCHASE)]
            chase_part = [smallp.tile([P, NSB], f32,
                                      name=f"part_0_{j}", tag="part")
                          for j in range(NCHASE)]
            for q in range(4):
                tq = [psump.tile([P, 2, SBLK], f32,
                                 name=f"ps_0_{j}_{q}", tag="ps")
                      for j in range(NCHASE)]
                for hi in range(HT):
                    for col in range(2):
                        for j in range(NCHASE):
                            si = 2 * q + col
                            nc.tensor.matmul(
                                tq[j][:, col],
                                lhsT=hid_tiles[0, j][:, hi, :],
                                rhs=enc_tiles[0, hi][:, si * SBLK:
                                                     (si + 1) * SBLK],
                                start=hi == 0,
                                stop=hi == HT - 1,
                            )
                for j in range(NCHASE):
                    nc.scalar.activation(tq[j], tq[j], Act.Tanh)
                    nc.scalar.activation(
                        chase_exp[j][:, 2 * q:2 * q + 2], tq[j], Act.Exp,
                        accum_out=chase_part[j][:, q:q + 1])
            for j in range(NCHASE):
                finalize(0, j, chase_exp[j], chase_part[j], 4, "steady")

            # ---- remaining t-tiles: steady state ----
            for b in range(B_LOC):
                for ti in range(NCHASE if b == 0 else 0, TT):
                    hid_t = hid_tiles[b, ti]
                    exp_row = expp.tile([P, NSB, SBLK], bf16,
                                        name=f"exp_{b}_{ti}", tag="exp")
                    last_tile = b == B_LOC - 1 and ti == TT - 1
                    mode = ("tail" if last_tile else
                            "pretail" if b == B_LOC - 1 and ti == TT - 2
                            else "steady")

                    pss = [psump.tile([P, 2, SBLK], f32,
                                      name=f"ps_{b}_{ti}_{sp}", tag="ps")
                           for sp in range(NMB)]
                    if last_tile:
                        # si7 gets its own psum tile: sharing a tile with
                        # si6 would make si6's tanh wait on si7's matmuls
                        # (tile-granular WAR tracking), serializing ~1.5us
                        # of extra ACT after the final matmul.
                        ps7 = psump.tile([P, 2, SBLK], f32,
                                         name=f"ps_{b}_{ti}_s7", tag="ps")

                    for si in range(NSB):
                        tgt = (ps7[:, 0] if last_tile and si == NSB - 1
                               else pss[si // 2][:, si % 2])
                        for hi in range(HT):
                            nc.tensor.matmul(
                                tgt,
                                lhsT=hid_t[:, hi, :],
                                rhs=enc_tiles[b, hi][:, si * SBLK:
                                                     (si + 1) * SBLK],
                                start=hi == 0,
                                stop=hi == HT - 1,
                            )

                    partials = smallp.tile([P, NSB], f32,
                                           name=f"part_{b}_{ti}", tag="part")
                    if last_tile:
                        n_acc = NMB + 1
                        for sp in range(NMB - 1):
                            nc.scalar.activation(pss[sp], pss[sp], Act.Tanh)
                            nc.scalar.activation(
                                exp_row[:, 2 * sp:2 * sp + 2], pss[sp],
                                Act.Exp,
                                accum_out=partials[:, sp:sp + 1])
                        for col in range(2):
                            si = 2 * (NMB - 1) + col
                            blk = (ps7[:, 0] if si == NSB - 1
                                   else pss[NMB - 1][:, col])
                            nc.scalar.activation(blk, blk, Act.Tanh)
                            nc.scalar.activation(
                                exp_row[:, si], blk, Act.Exp,
                                accum_out=partials[:, NMB - 1 + col:
                                                   NMB + col])
                    else:
                        n_acc = NMB
                        for sp in range(NMB):
                            nc.scalar.activation(pss[sp], pss[sp], Act.Tanh)
                            nc.scalar.activation(
                                exp_row[:, 2 * sp:2 * sp + 2], pss[sp],
                                Act.Exp,
                                accum_out=partials[:, sp:sp + 1])

                    finalize(b, ti, exp_row, partials, n_acc, mode)
    nc.compile()
    return nc


def kernel(hiddenState: np.ndarray, encoderOut: np.ndarray) -> np.ndarray:
    import ml_dtypes
    from concourse import bass_utils

    hiddenState = np.asarray(hiddenState, dtype=np.float32)
    encoderOut = np.asarray(encoderOut, dtype=np.float32)

    # [TQ, B, H] -> [B, H, TQ] -> [B, HT, P(hp), TT, P(t)]
    #            -> [B, TT, P(hp), HT, P(t)]  (contiguous 2KB per partition)
    hidT = np.ascontiguousarray(
        hiddenState.transpose(1, 2, 0)
        .reshape(B, HT, P, TT, P)
        .transpose(0, 3, 2, 1, 4)
    ).astype(ml_dtypes.bfloat16)
    # [S, B, H] -> [B, HT, P, S]
    encT = np.ascontiguousarray(encoderOut.transpose(1, 2, 0)).reshape(
        B, HT, P, S).astype(ml_dtypes.bfloat16)

    if "nc" not in _CACHE:
        _CACHE["nc"] = _build()
    nc = _CACHE["nc"]

    in_maps = [
        {"hidT": hidT[c * B_LOC:(c + 1) * B_LOC],
         "encT": encT[c * B_LOC:(c + 1) * B_LOC]}
        for c in range(NCORES)
    ]
    res = bass_utils.run_bass_kernel_spmd(
        nc, in_maps, core_ids=list(range(NCORES)))
    _CACHE["last_results"] = res

    # per-core [B_LOC, TT, P, S] bf16 -> full [B, TQ, S] -> [B, S, TQ] f32
    out = np.concatenate([r["attW"] for r in res.results], axis=0)
    out = out.reshape(B, TQ, S).transpose(0, 2, 1).astype(np.float32)
    return np.ascontiguousarray(out)
